# revision 2
# baseline (speedup 1.0000x reference)
"""Trainium2 Bass kernel for nn_GatedMultiHeadGATLayer (gnn_message_passing), v3.

Strategy (8 NeuronCores, SPMD single NEFF):
- Nodes remapped (degree-stratified round-robin) into 320 blocks of 128; cores
  own 40 consecutive blocks (5120 nodes) and fully own the segment reductions
  for their dst range (edges sharded by dst block, no all-reduce).
- Per dst block, edges split by src half (lo/hi) for int16 dma_gather indices,
  padded (idx 0, pde 0, dst 128) to a uniform chunk grid.
- Layer tables are fp8e4 (z scaled x16) + f32 attention scalars packed in
  768B (L1) / 256B (L2) rows, AllGathered between phases.
- Selection matrices (edge->dst one-hot, both orientations) are precomputed on
  the host as fp8 0/1 inputs, so the device does no iota compares or PE
  broadcast transposes; attention scalars come from DVE tensor_tensor_reduce
  against host-replicated attention rows (no per-block PE transposes at all).
- The scalar engine runs (almost) only Exp: leaky-relu runs on DVE via
  scalar_tensor_tensor max(x, 0.01x); GRU sigmoid/tanh are computed from Exp +
  DVE reciprocal, so activation-table reloads stay off the critical chain.
- Block loops are software-pipelined (pre-gather work of block b+1 is emitted
  before the gather-dependent work of block b) so the in-order engine queues
  never head-block on an in-flight gather.
"""
import sys

sys.path.insert(0, "/opt/trn_rl_repo")

import numpy as np

import concourse.bass as bass
import concourse.bacc as bacc
import concourse.tile as tile
import concourse.mybir as mybir
from concourse import bass_utils

N = 40000
E = 640000
DIM = 128
HEADS = 4
NCORES = 8
TOTB = 320
B = TOTB // NCORES
PN = B * 128
NP = TOTB * 128
HALF = NP // 2
ROW1 = 768            # fp8 bytes: [asrc4 f32 (16B) | z 512 fp8 | pad]
ROW2 = 256            # fp8 bytes: [a2src f32 (4B) | z2 128 fp8 | pad]
ZSCALE = 1.0          # z stored as fp8e4 (plain)

import os as _os
f32 = mybir.dt.float32
bf16 = mybir.dt.bfloat16
fp8 = mybir.dt.float8e4
SELFP8 = _os.environ.get("GAT_SELFP8", "1") == "1"
seldt = fp8 if SELFP8 else bf16
i16 = mybir.dt.int16
AF = mybir.ActivationFunctionType
ALU = mybir.AluOpType


def _pack_edges(src, dst, pd, o2n, ew, ew2):
    # ew/ew2 arrive pre-multiplied by mw/mw2 (lrelu is positively homogeneous)
    import ml_dtypes
    nsrc = o2n[src]
    ndst = o2n[dst]
    eblk = ndst >> 7
    hi = (nsrc >= HALF).astype(np.int64)
    key = eblk * 2 + hi
    sidx = np.argsort(key, kind="stable")
    skey = key[sidx]
    gcnt = np.bincount(key, minlength=TOTB * 2)
    K_lo = int(-(-gcnt[0::2].max() // 128))
    K_hi = int(-(-gcnt[1::2].max() // 128))
    gstart = np.zeros(TOTB * 2, np.int64)
    gstart[1:] = np.cumsum(gcnt)[:-1]
    pos = np.arange(E) - gstart[skey]

    def grid(S, msel):
        idx = np.zeros((TOTB, S), np.int32)
        dr = np.full((TOTB, S), 128.0, np.float32)
        pde = np.zeros((TOTB, S), np.float32)
        eb = skey[msel] // 2
        p = pos[msel]
        e = sidx[msel]
        idx[eb, p] = nsrc[e]
        dr[eb, p] = (ndst[e] - eb * 128).astype(np.float32)
        pde[eb, p] = pd[e, 0]
        return idx, dr, pde

    m_lo = (skey % 2) == 0
    idx_lo, dr_lo, pd_lo = grid(K_lo * 128, m_lo)
    idx_hi, dr_hi, pd_hi = grid(K_hi * 128, ~m_lo)
    idx_hi[:] = np.where(idx_hi > 0, idx_hi - HALF, 0)

    def chunks(a, S):  # [TOTB, S] -> [128, TOTB, S//128], slot (p,j)=list j*128+p
        return np.ascontiguousarray(a.reshape(TOTB, S // 128, 128).transpose(2, 0, 1))

    def pack16(a, S):  # int16 gather-index layout: elem k at [k%16, k//16], x8
        b = a.astype(np.int16).reshape(TOTB, S // 16, 16).transpose(2, 0, 1)
        return np.ascontiguousarray(np.tile(b, (8, 1, 1)))

    K = K_lo + K_hi
    dstr = np.concatenate([chunks(dr_lo, K_lo * 128), chunks(dr_hi, K_hi * 128)], 2)
    pde = np.concatenate([chunks(pd_lo, K_lo * 128), chunks(pd_hi, K_hi * 128)], 2)
    # selection matrices as fp8 0/1, both orientations
    nids = np.arange(128, dtype=np.float32)
    _sdt = ml_dtypes.float8_e4m3 if SELFP8 else ml_dtypes.bfloat16
    selH = (dstr[:, :, :, None] == nids).astype(_sdt)
    drow = np.concatenate(
        [dr_lo.reshape(TOTB, K_lo * 128), dr_hi.reshape(TOTB, K_hi * 128)], 1)
    selTH = (nids[:, None, None] == drow[None, :, :]).astype(_sdt)
    pde4 = pde[:, :, :, None] * np.asarray(ew, np.float32)[None, None, None, :]
    pde2 = pde * np.float32(ew2)
    return (pack16(idx_lo, K_lo * 128), pack16(idx_hi, K_hi * 128),
            np.ascontiguousarray(selH.reshape(128, TOTB, K * 128)),
            np.ascontiguousarray(selTH),
            np.ascontiguousarray(pde4, np.float32).reshape(128, TOTB, K * HEADS),
            np.ascontiguousarray(pde2, np.float32), K_lo, K_hi)


def _build_nc(K_lo, K_hi, mw2):
    import os
    _phases = int(os.environ.get("GAT_PHASES", "2"))
    K = K_lo + K_hi
    nc = bacc.Bacc("TRN2", target_bir_lowering=False, debug=False,
                   num_devices=NCORES)
    # ---- I/O ----
    h_sl = nc.dram_tensor("h_sl", [PN, DIM], f32, kind="ExternalInput")
    hTb_i = nc.dram_tensor("hTb", [128, PN], bf16, kind="ExternalInput")
    idxlo = nc.dram_tensor("idxlo", [128, B, 8 * K_lo], i16, kind="ExternalInput")
    idxhi = nc.dram_tensor("idxhi", [128, B, 8 * K_hi], i16, kind="ExternalInput")
    sel_i = nc.dram_tensor("sel", [128, B, K * 128], seldt, kind="ExternalInput")
    selT_i = nc.dram_tensor("selT", [128, B, K * 128], seldt, kind="ExternalInput")
    pde4_i = nc.dram_tensor("pde4", [128, B, K * HEADS], f32, kind="ExternalInput")
    pde2_i = nc.dram_tensor("pde2", [128, B, K], f32, kind="ExternalInput")
    ident_i = nc.dram_tensor("ident", [128, 128], f32, kind="ExternalInput")
    onesc_i = nc.dram_tensor("onesc", [128, 1], f32, kind="ExternalInput")
    fcWT_i = nc.dram_tensor("fcWT", [128, 4 * DIM], bf16, kind="ExternalInput")
    WT2_i = nc.dram_tensor("WT2", [128, 4 * DIM], bf16, kind="ExternalInput")
    attnp_i = nc.dram_tensor("attnp", [128, 8], bf16, kind="ExternalInput")
    attn2_i = nc.dram_tensor("attn2", [128, 2], bf16, kind="ExternalInput")
    WihT_i = nc.dram_tensor("WihT", [128, 3 * DIM], bf16, kind="ExternalInput")
    WhhT_i = nc.dram_tensor("WhhT", [128, 3 * DIM], bf16, kind="ExternalInput")
    bih_i = nc.dram_tensor("bih", [128, 3 * DIM], f32, kind="ExternalInput")
    bhh_i = nc.dram_tensor("bhh", [128, 3 * DIM], f32, kind="ExternalInput")
    out_sl = nc.dram_tensor("out_sl", [PN, DIM], f32, kind="ExternalOutput")
    # ---- internal DRAM ----
    Tz1_sl = nc.dram_tensor("Tz1_sl", [PN, ROW1], fp8, kind="Internal")
    Tz1 = nc.dram_tensor("Tz1", [NP, ROW1], fp8, kind="Internal",
                         addr_space="Shared")
    T2_sl = nc.dram_tensor("T2_sl", [PN, ROW2], fp8, kind="Internal")
    T2 = nc.dram_tensor("T2", [NP, ROW2], fp8, kind="Internal",
                        addr_space="Shared")
    Adst = nc.dram_tensor("Adst", [PN, 4], bf16, kind="Internal")
    A2dst = nc.dram_tensor("A2dst", [PN, 1], bf16, kind="Internal")

    rg = [list(range(NCORES))]
    with tile.TileContext(nc) as tc:
        with (
            tc.tile_pool(name="const", bufs=1) as cp,
            tc.tile_pool(name="ed", bufs=1) as edp,
            tc.tile_pool(name="zg", bufs=6) as zgp,
            tc.tile_pool(name="work", bufs=3) as wp,
            tc.tile_pool(name="sel", bufs=4) as selp,
            tc.tile_pool(name="psz", bufs=3, space="PSUM") as psz,
            tc.tile_pool(name="pssm", bufs=3, space="PSUM") as pssm,
            tc.tile_pool(name="pstp", bufs=2, space="PSUM") as pstp,
        ):
            def cload(t_in, shape, dtype):
                t = cp.tile(shape, dtype, tag=t_in.name)
                nc.sync.dma_start(out=t[:], in_=t_in[(slice(None),) * len(shape)])
                return t

            ident = cload(ident_i, [128, 128], f32)
            identb = wp.tile([128, 128], bf16, tag="identb")
            nc.vector.tensor_copy(out=identb[:], in_=ident[:])
            onesc = cload(onesc_i, [128, 1], f32)
            fcWT = cload(fcWT_i, [128, 4 * DIM], bf16)
            WT2 = cload(WT2_i, [128, 4 * DIM], bf16)
            attnp = cload(attnp_i, [128, 8], bf16)
            attn2 = cload(attn2_i, [128, 2], bf16)
            WihT = cload(WihT_i, [128, 3 * DIM], bf16)
            WhhT = cload(WhhT_i, [128, 3 * DIM], bf16)
            bih = cload(bih_i, [128, 3 * DIM], f32)
            bhh = cload(bhh_i, [128, 3 * DIM], f32)
            idxlo_t = edp.tile([128, B, 8 * K_lo], i16)
            nc.sync.dma_start(out=idxlo_t[:, :, :], in_=idxlo[:, :, :])
            idxhi_t = edp.tile([128, B, 8 * K_hi], i16)
            nc.sync.dma_start(out=idxhi_t[:, :, :], in_=idxhi[:, :, :])
            pde4_t = edp.tile([128, B, K * HEADS], f32)
            nc.sync.dma_start(out=pde4_t[:, :, :], in_=pde4_i[:, :, :])
            pde2_t = edp.tile([128, B, K], f32)
            nc.sync.dma_start(out=pde2_t[:, :, :], in_=pde2_i[:, :, :])

            stt = nc.vector.scalar_tensor_tensor

            def lrelu_dve(out_ap, in_ap):
                stt(out=out_ap, in0=in_ap, scalar=0.01, in1=in_ap,
                    op0=ALU.mult, op1=ALU.max)

            # ================= phase 0: per-node z/asrc/adst =================
            def ph_pre(b):
                rows = slice(b * 128, (b + 1) * 128)
                hTb = wp.tile([128, 128], bf16, tag="hTb")
                nc.sync.dma_start(out=hTb[:], in_=hTb_i[:, rows])
                pz = psz.tile([128, 512], f32, space="PSUM", tag="z")
                nc.tensor.matmul(pz[:], lhsT=hTb[:], rhs=fcWT[:], start=True,
                                 stop=True)
                pzT = pstp.tile([128, 512], f32, space="PSUM", tag="tp")
                for h4 in range(HEADS):
                    nc.tensor.matmul(pzT[:, h4 * 128:(h4 + 1) * 128],
                                     lhsT=fcWT[:, h4 * 128:(h4 + 1) * 128],
                                     rhs=hTb[:], start=True, stop=True)
                return dict(b=b, pz=pz, pzT=pzT)

            def ph_post(c):
                b = c["b"]
                rows = slice(b * 128, (b + 1) * 128)
                t1row = wp.tile([128, ROW1], fp8, tag="t1row")
                nc.scalar.activation(t1row[:, 16:528], c["pz"][:], AF.Lrelu,
                                     alpha=0.01)
                zT = wp.tile([128, 512], bf16, tag="zT")
                nc.scalar.activation(zT[:], c["pzT"][:], AF.Lrelu, alpha=0.01)
                sm = pssm.tile([128, 128], f32, space="PSUM", tag="sm")
                a8 = sm[:, 0:8]
                for h4 in range(HEADS):
                    nc.tensor.matmul(a8[:, 2 * h4:2 * h4 + 2],
                                     lhsT=zT[:, h4 * 128:(h4 + 1) * 128],
                                     rhs=attnp[:, 2 * h4:2 * h4 + 2],
                                     start=True, stop=True)
                a8v = a8.rearrange("p (four two) -> p four two", two=2)
                nc.vector.tensor_copy(out=t1row[:, 0:16].bitcast(f32),
                                      in_=a8v[:, :, 0])
                adb = wp.tile([128, 4], bf16, tag="adb")
                nc.vector.tensor_copy(out=adb[:], in_=a8v[:, :, 1])
                nc.sync.dma_start(out=Adst[rows, :], in_=adb[:])
                nc.sync.dma_start(out=Tz1_sl[rows, :], in_=t1row[:])

            pctx = None
            for b in range(B):
                npc = ph_pre(b)
                if pctx is not None:
                    ph_post(pctx)
                pctx = npc
            ph_post(pctx)

            nc.gpsimd.collective_compute(
                "AllGather", ALU.bypass, replica_groups=rg,
                ins=[Tz1_sl[:, :]], outs=[Tz1[:, :]])

            def recip_scaled(pss, nh):
                den = wp.tile([128, nh], f32, tag="den")
                nc.vector.tensor_scalar(out=den[:], in0=pss, scalar1=1e-30,
                                        scalar2=None, op0=ALU.max)
                r = wp.tile([128, nh], f32, tag="rcp")
                nc.vector.reciprocal(out=r[:], in_=den[:])
                return r

            # ================= layer 1 (software-pipelined) ==================
            def l1_pre(b):
                rows = slice(b * 128, (b + 1) * 128)
                zlo = zgp.tile([128, K_lo, ROW1], fp8, tag="zlo")
                zhi = zgp.tile([128, K_hi, ROW1], fp8, tag="zhi")
                nc.gpsimd.dma_gather(
                    out_ap=zlo[:, :, :], in_ap=Tz1[:, :],
                    idxs_ap=idxlo_t[:, b, :], num_idxs=K_lo * 128,
                    num_idxs_reg=K_lo * 128, elem_size=ROW1, single_packet=False)
                nc.gpsimd.dma_gather(
                    out_ap=zhi[:, :, :], in_ap=Tz1[HALF:, :],
                    idxs_ap=idxhi_t[:, b, :], num_idxs=K_hi * 128,
                    num_idxs_reg=K_hi * 128, elem_size=ROW1, single_packet=False)
                sel_t = selp.tile([128, K, 128], seldt, tag="sel")
                nc.sync.dma_start(out=sel_t[:].rearrange("p k e -> p (k e)"),
                                  in_=sel_i[:, b, :])
                selT_t = selp.tile([128, K, 128], seldt, tag="selT")
                nc.sync.dma_start(out=selT_t[:].rearrange("p k e -> p (k e)"),
                                  in_=selT_i[:, b, :])
                adb = wp.tile([128, 4], bf16, tag="adbl")
                nc.sync.dma_start(out=adb[:], in_=Adst[rows, :])
                return dict(b=b, zlo=zlo, zhi=zhi, sel=sel_t, selT=selT_t,
                            adb=adb)

            def l1_mid(c):
                b = c["b"]
                rows = slice(b * 128, (b + 1) * 128)
                zlo, zhi, sel_t = c["zlo"], c["zhi"], c["sel"]
                sm = pssm.tile([128, 128], f32, space="PSUM", tag="sm")
                c["sm"] = sm
                adps_v = sm[:, 0:K * HEADS].rearrange("p (k h) -> p k h",
                                                      h=HEADS)
                for j in range(K):
                    nc.tensor.matmul(adps_v[:, j, :], lhsT=c["selT"][:, j, :],
                                     rhs=c["adb"][:], start=True, stop=True)
                c["adps_v"] = adps_v
                asr = wp.tile([128, K, HEADS], f32, tag="asr")
                nc.vector.tensor_tensor(out=asr[:, :K_lo, :],
                                        in0=zlo[:, :, 0:16].bitcast(f32),
                                        in1=c["adps_v"][:, :K_lo, :],
                                        op=ALU.add)
                nc.vector.tensor_tensor(out=asr[:, K_lo:, :],
                                        in0=zhi[:, :, 0:16].bitcast(f32),
                                        in1=c["adps_v"][:, K_lo:, :],
                                        op=ALU.add)
                nc.vector.tensor_tensor(
                    out=asr[:, :, :], in0=asr[:, :, :],
                    in1=pde4_t[:, b, :].rearrange("p (k h) -> p k h", h=HEADS),
                    op=ALU.mult)
                exf = wp.tile([128, K, HEADS], f32, tag="exf")
                av = asr[:].rearrange("p k h -> p (k h)")
                ev = exf[:].rearrange("p k h -> p (k h)")
                lrelu_dve(ev, av)
                ex = wp.tile([128, K, HEADS], bf16, tag="ex")
                nc.scalar.activation(ex[:].rearrange("p k h -> p (k h)"),
                                     ev, AF.Exp)
                pagg = psz.tile([128, 512], f32, space="PSUM", tag="z")
                pss = sm[:, K * HEADS:K * HEADS + HEADS]
                for j in range(K):
                    zg_, jj = (zlo, j) if j < K_lo else (zhi, j - K_lo)
                    zgs = wp.tile([128, 512], bf16, tag="zgs")
                    nc.vector.tensor_tensor(
                        out=zgs[:].rearrange("p (h d) -> p h d", d=128),
                        in0=zg_[:, jj, 16:528].rearrange("p (h d) -> p h d",
                                                         d=128),
                        in1=ex[:, j, :, None].to_broadcast([128, HEADS, 128]),
                        op=ALU.mult)
                    nc.tensor.matmul(pagg[:], lhsT=sel_t[:, j, :], rhs=zgs[:],
                                     start=(j == 0), stop=(j == K - 1))
                    nc.tensor.matmul(pss, lhsT=sel_t[:, j, :], rhs=ex[:, j, :],
                                     start=(j == 0), stop=(j == K - 1))
                c["pagg"] = pagg
                c["pss"] = pss

            def l1_epi(c):
                b = c["b"]
                rows = slice(b * 128, (b + 1) * 128)
                pagg, pss = c["pagg"], c["pss"]
                r4 = recip_scaled(pss, HEADS)
                xb = wp.tile([128, 512], bf16, tag="xb")
                for h4 in range(HEADS):
                    nc.scalar.activation(xb[:, h4 * 128:(h4 + 1) * 128],
                                         pagg[:, h4 * 128:(h4 + 1) * 128],
                                         AF.Lrelu, scale=r4[:, h4:h4 + 1],
                                         alpha=0.01)
                xT = wp.tile([128, 512], bf16, tag="xT")
                for q in range(4):
                    tpq = pstp.tile([128, 512], bf16, space="PSUM", tag="tp")
                    nc.tensor.transpose(out=tpq[:, 0:128],
                                        in_=xb[:, q * 128:(q + 1) * 128],
                                        identity=identb[:])
                    nc.vector.tensor_copy(out=xT[:, q * 128:(q + 1) * 128],
                                          in_=tpq[:, 0:128])
                pz2 = psz.tile([128, 128], f32, space="PSUM", tag="z")
                pz2T = pstp.tile([128, 512], f32, space="PSUM", tag="tp")
                for q in range(4):
                    nc.tensor.matmul(pz2[:], lhsT=xT[:, q * 128:(q + 1) * 128],
                                     rhs=WT2[:, q * 128:(q + 1) * 128],
                                     start=(q == 0), stop=(q == 3))
                    nc.tensor.matmul(pz2T[:, 0:128],
                                     lhsT=WT2[:, q * 128:(q + 1) * 128],
                                     rhs=xT[:, q * 128:(q + 1) * 128],
                                     start=(q == 0), stop=(q == 3))
                t2row = wp.tile([128, ROW2], fp8, tag="t2row")
                nc.scalar.activation(t2row[:, 4:132], pz2[:], AF.Lrelu,
                                     alpha=0.01)
                z2T = wp.tile([128, 128], bf16, tag="z2T")
                nc.scalar.activation(z2T[:], pz2T[:, 0:128], AF.Lrelu,
                                     alpha=0.01)
                a2 = c["sm"][:, K * HEADS + HEADS:K * HEADS + HEADS + 2]
                nc.tensor.matmul(a2, lhsT=z2T[:], rhs=attn2[:],
                                 start=True, stop=True)
                nc.vector.tensor_copy(out=t2row[:, 0:4].bitcast(f32),
                                      in_=a2[:, 0:1])
                a2d = wp.tile([128, 1], bf16, tag="a2d")
                nc.vector.tensor_copy(out=a2d[:], in_=a2[:, 1:2])
                nc.sync.dma_start(out=A2dst[rows, :], in_=a2d[:])
                nc.sync.dma_start(out=T2_sl[rows, :], in_=t2row[:])

            if _phases >= 1:
                ctxs = []
                for b in range(B):
                    ctxs.append(l1_pre(b))
                    if len(ctxs) >= 2:
                        l1_mid(ctxs[-2])
                    if len(ctxs) >= 3:
                        l1_epi(ctxs[-3])
                        ctxs[-3] = None
                l1_mid(ctxs[-1])
                l1_epi(ctxs[-2])
                l1_epi(ctxs[-1])

            if _phases >= 2:
                nc.gpsimd.collective_compute(
                    "AllGather", ALU.bypass, replica_groups=rg,
                    ins=[T2_sl[:, :]], outs=[T2[:, :]])

            # ================= layer 2 (software-pipelined, inline GRU) ======
            def l2_pre(b):
                rows = slice(b * 128, (b + 1) * 128)
                zlo = zgp.tile([128, K_lo, ROW2], fp8, tag="zlo2")
                zhi = zgp.tile([128, K_hi, ROW2], fp8, tag="zhi2")
                nc.gpsimd.dma_gather(
                    out_ap=zlo[:, :, :], in_ap=T2[:, :],
                    idxs_ap=idxlo_t[:, b, :], num_idxs=K_lo * 128,
                    num_idxs_reg=K_lo * 128, elem_size=ROW2, single_packet=False)
                nc.gpsimd.dma_gather(
                    out_ap=zhi[:, :, :], in_ap=T2[HALF:, :],
                    idxs_ap=idxhi_t[:, b, :], num_idxs=K_hi * 128,
                    num_idxs_reg=K_hi * 128, elem_size=ROW2, single_packet=False)
                sel_t = selp.tile([128, K, 128], seldt, tag="sel")
                nc.sync.dma_start(out=sel_t[:].rearrange("p k e -> p (k e)"),
                                  in_=sel_i[:, b, :])
                selT_t = selp.tile([128, K, 128], seldt, tag="selT")
                nc.sync.dma_start(out=selT_t[:].rearrange("p k e -> p (k e)"),
                                  in_=selT_i[:, b, :])
                adb2 = wp.tile([128, 1], bf16, tag="adb2")
                nc.sync.dma_start(out=adb2[:], in_=A2dst[rows, :])
                hb = wp.tile([128, 128], f32, tag="hblk")
                nc.sync.dma_start(out=hb[:], in_=h_sl[rows, :])
                hTb = wp.tile([128, 128], bf16, tag="hTb2")
                nc.sync.dma_start(out=hTb[:], in_=hTb_i[:, rows])
                return dict(b=b, zlo=zlo, zhi=zhi, sel=sel_t, selT=selT_t,
                            adb2=adb2, hb=hb, hTb=hTb)

            def l2_mid(c):
                b = c["b"]
                rows = slice(b * 128, (b + 1) * 128)
                zlo, zhi, sel_t = c["zlo"], c["zhi"], c["sel"]
                sm = pssm.tile([128, 128], f32, space="PSUM", tag="sm")
                c["sm"] = sm
                adps = sm[:, 0:K]
                for j in range(K):
                    nc.tensor.matmul(adps[:, j:j + 1], lhsT=c["selT"][:, j, :],
                                     rhs=c["adb2"][:], start=True, stop=True)
                c["adps"] = adps
                asr = wp.tile([128, K], f32, tag="asr2")
                nc.vector.tensor_tensor(out=asr[:, :K_lo],
                                        in0=zlo[:, :, 0:4].bitcast(f32)[:, :, 0],
                                        in1=c["adps"][:, :K_lo], op=ALU.add)
                nc.vector.tensor_tensor(out=asr[:, K_lo:],
                                        in0=zhi[:, :, 0:4].bitcast(f32)[:, :, 0],
                                        in1=c["adps"][:, K_lo:], op=ALU.add)
                nc.vector.tensor_tensor(out=asr[:], in0=asr[:],
                                        in1=pde2_t[:, b, :], op=ALU.mult)
                exf = wp.tile([128, K], f32, tag="ex2f")
                lrelu_dve(exf[:], asr[:])
                ex2 = wp.tile([128, K], bf16, tag="ex2")
                nc.scalar.activation(ex2[:], exf[:], AF.Exp)
                pagg = psz.tile([128, 128], f32, space="PSUM", tag="z")
                pss2 = sm[:, K:K + 1]
                for j in range(K):
                    zg_, jj = (zlo, j) if j < K_lo else (zhi, j - K_lo)
                    zgs = wp.tile([128, 128], bf16, tag="zgs2")
                    nc.vector.tensor_tensor(
                        out=zgs[:], in0=zg_[:, jj, 4:132],
                        in1=ex2[:, j:j + 1].to_broadcast([128, 128]),
                        op=ALU.mult)
                    nc.tensor.matmul(pagg[:], lhsT=sel_t[:, j, :], rhs=zgs[:],
                                     start=(j == 0), stop=(j == K - 1))
                    nc.tensor.matmul(pss2, lhsT=sel_t[:, j, :],
                                     rhs=ex2[:, j:j + 1], start=(j == 0),
                                     stop=(j == K - 1))
                c["pagg"] = pagg
                c["pss2"] = pss2

            def l2_epi(c):
                b = c["b"]
                rows = slice(b * 128, (b + 1) * 128)
                pagg, pss2 = c["pagg"], c["pss2"]
                r1 = recip_scaled(pss2, 1)
                x2 = wp.tile([128, 128], bf16, tag="x2")
                nc.scalar.activation(x2[:], pagg[:], AF.Lrelu,
                                     scale=r1[:, 0:1], alpha=0.01)
                tpx = pstp.tile([128, 512], bf16, space="PSUM", tag="tp")
                nc.tensor.transpose(out=tpx[:, 0:128], in_=x2[:],
                                    identity=identb[:])
                x2T = wp.tile([128, 128], bf16, tag="x2T")
                nc.vector.tensor_copy(out=x2T[:], in_=tpx[:, 0:128])
                gips = psz.tile([128, 384], f32, space="PSUM", tag="z")
                nc.tensor.matmul(gips[:], lhsT=x2T[:], rhs=WihT[:],
                                 start=True, stop=True)
                ghps = pstp.tile([128, 384], f32, space="PSUM", tag="tp")
                nc.tensor.matmul(ghps[:], lhsT=c["hTb"][:], rhs=WhhT[:],
                                 start=True, stop=True)
                gi = wp.tile([128, 384], f32, tag="gi")
                nc.vector.tensor_tensor(out=gi[:], in0=gips[:], in1=bih[:],
                                        op=ALU.add)
                gh = wp.tile([128, 384], f32, tag="gh")
                nc.vector.tensor_tensor(out=gh[:], in0=ghps[:], in1=bhh[:],
                                        op=ALU.add)
                rz = wp.tile([128, 256], f32, tag="rz")
                nc.vector.tensor_tensor(out=rz[:], in0=gi[:, 0:256],
                                        in1=gh[:, 0:256], op=ALU.add)
                nc.scalar.activation(rz[:], rz[:], AF.Sigmoid)
                nt = wp.tile([128, 128], f32, tag="nt")
                nc.vector.tensor_tensor(out=nt[:], in0=rz[:, 0:128],
                                        in1=gh[:, 256:384], op=ALU.mult)
                nc.vector.tensor_tensor(out=nt[:], in0=nt[:],
                                        in1=gi[:, 256:384], op=ALU.add)
                nc.scalar.activation(nt[:], nt[:], AF.Tanh)
                o1 = wp.tile([128, 128], f32, tag="o1")
                nc.vector.tensor_tensor(out=o1[:], in0=rz[:, 128:256],
                                        in1=nt[:], op=ALU.mult)
                nc.vector.tensor_tensor(out=nt[:], in0=nt[:], in1=o1[:],
                                        op=ALU.subtract)
                nc.vector.tensor_tensor(out=o1[:], in0=rz[:, 128:256],
                                        in1=c["hb"][:], op=ALU.mult)
                nc.vector.tensor_tensor(out=nt[:], in0=nt[:], in1=o1[:],
                                        op=ALU.add)
                nc.scalar.activation(nt[:], nt[:], AF.Lrelu, alpha=0.01)
                nc.sync.dma_start(out=out_sl[rows, :], in_=nt[:])

            if _phases >= 2:
                ctxs = []
                for b in range(B):
                    ctxs.append(l2_pre(b))
                    if len(ctxs) >= 2:
                        l2_mid(ctxs[-2])
                    if len(ctxs) >= 3:
                        l2_epi(ctxs[-3])
                        ctxs[-3] = None
                l2_mid(ctxs[-1])
                l2_epi(ctxs[-2])
                l2_epi(ctxs[-1])
    nc.finalize()
    return nc


def _bf(x):
    import ml_dtypes
    return np.asarray(np.asarray(x, np.float32), ml_dtypes.bfloat16)


def kernel(h, pd, fc_W, attn_W, edge_w, m_w, out_fc_W, out_attn_W, out_edge_w,
           out_m_w, gru_Wih, gru_Whh, gru_bih, gru_bhh, src, dst):
    h = np.asarray(h, np.float32)
    pd = np.asarray(pd, np.float32)
    src = np.asarray(src, np.int64)
    dst = np.asarray(dst, np.int64)
    deg = np.bincount(dst, minlength=N)
    order = np.argsort(-deg, kind="stable")
    o2n = np.empty(N, np.int64)
    o2n[order] = (np.arange(N) % TOTB) * 128 + np.arange(N) // TOTB
    ew = [float(edge_w[i, 0, 0]) for i in range(HEADS)]
    mw = [float(m_w[i, 0, 0]) for i in range(HEADS)]
    assert all(m > 0 for m in mw) and float(out_m_w[0, 0]) > 0
    ewm = [ew[i] * mw[i] for i in range(HEADS)]
    ilo, ihi, selH, selTH, pde4, pde2, K_lo, K_hi = _pack_edges(
        src, dst, pd, o2n, ewm, float(out_edge_w[0, 0]) * float(out_m_w[0, 0]))

    h_new = np.zeros((NP, DIM), np.float32)
    h_new[o2n] = h
    hT = np.ascontiguousarray(h_new.T)                       # [128, NP]
    fcWT = np.concatenate([np.asarray(fc_W[i], np.float32).T
                           for i in range(HEADS)], 1)
    WT2 = np.ascontiguousarray(
        np.asarray(out_fc_W, np.float32).reshape(DIM, 4, DIM)
        .transpose(2, 1, 0).reshape(128, 512))
    attnp = np.zeros((128, 8), np.float32)
    for i in range(HEADS):
        attnp[:, 2 * i] = attn_W[i, 0, :DIM]
        attnp[:, 2 * i + 1] = attn_W[i, 0, DIM:]
    attn2 = np.stack([out_attn_W[0, :DIM], out_attn_W[0, DIM:]], 1)
    consts = {
        "ident": np.eye(128, dtype=np.float32),
        "onesc": np.ones((128, 1), np.float32),
        "fcWT": _bf(fcWT), "WT2": _bf(WT2),
        "attnp": _bf(attnp), "attn2": _bf(attn2),
        "WihT": _bf(np.asarray(gru_Wih, np.float32).T),
        "WhhT": _bf(np.asarray(gru_Whh, np.float32).T),
        "bih": np.tile(np.asarray(gru_bih, np.float32)[None, :], (128, 1)),
        "bhh": np.tile(np.asarray(gru_bhh, np.float32)[None, :], (128, 1)),
    }
    nc = _build_nc(K_lo, K_hi, float(out_m_w[0, 0]))
    in_maps = []
    for c in range(NCORES):
        bs = slice(B * c, B * (c + 1))
        rs = slice(PN * c, PN * (c + 1))
        in_maps.append({
            "h_sl": np.ascontiguousarray(h_new[rs]),
            "hTb": _bf(hT[:, rs]),
            "idxlo": np.ascontiguousarray(ilo[:, bs, :]),
            "idxhi": np.ascontiguousarray(ihi[:, bs, :]),
            "sel": np.ascontiguousarray(selH[:, bs, :]),
            "selT": np.ascontiguousarray(selTH[:, bs, :]),
            "pde4": np.ascontiguousarray(pde4[:, bs, :]),
            "pde2": np.ascontiguousarray(pde2[:, bs, :]),
            **consts,
        })
    res = bass_utils.run_bass_kernel_spmd(nc, in_maps,
                                          core_ids=list(range(NCORES)))
    global _last_results
    _last_results = res
    out_new = np.concatenate([res.results[c]["out_sl"] for c in range(NCORES)])
    return np.ascontiguousarray(out_new[o2n])


_last_results = None
